# revision 53
# baseline (speedup 1.0000x reference)
"""DiT block (adaLN) Trainium2 kernel, 8-core SPMD, no collectives.

Sharding: core c handles batch b = c//2 and query-token half c%2 (1024 q
tokens).  Each core computes K/V for all 2048 tokens of its batch (the
only duplicated work), so cores never communicate.  The host permutes
each core's token columns so its own 1024 tokens come first (softmax is
invariant to key order), and transposes x to feature-major [D, L] so the
device never transposes anything.

On-device layout is feature-major everywhere: activations live as
[128 partitions, d-chunk, tokens].  LayerNorm stats (per-token = free
dim) are computed with ones-vector matmuls on the tensor engine and
broadcast back across partitions on GpSimd.  All GEMM operands are bf16
(fp32 PSUM accumulation); the residual stream, softmax and LN statistics
stay fp32.

The QKV projection and attention are fused into one software-pipelined
stream over head-chunks: softmax exp saturates the scalar engine
(~37us/chunk), so the in-order tensor engine is kept busy by hand-
interleaving independent work (next chunk's q/k projections, previous
tile's AV matmuls) between the exp-gated score matmuls.  Weights are
DMA'd in large row-contiguous tiles and sliced on-chip.
"""

import os
import sys
from collections import deque
from contextlib import ExitStack

os.environ.setdefault("MYCRO_LOCAL_CACHE", "1")
for _p in ("/opt/trn_rl_repo", "/root/.axon_site/_ro/trn_rl_repo"):
    if os.path.isdir(_p) and _p not in sys.path:
        sys.path.insert(0, _p)

import ml_dtypes
import numpy as np

import concourse.bass as bass
import concourse.tile as tile
from concourse import bacc, mybir
from concourse.bass_utils import run_bass_kernel_spmd

B, L, D, H, HD, MLPD = 4, 2048, 1024, 16, 64, 4096
NCORES = 8
LOWN = L // 2          # own query tokens per core
DC = D // 128          # 8 chunks of the model dim
MC = MLPD // 128       # 32 chunks of the mlp dim
LT = 512               # token tile for matmul free dim
NLT_OWN = LOWN // LT   # 2 token tiles (queries)
NHC = H // 2           # head-pair chunks

f32 = mybir.dt.float32
bf16 = mybir.dt.bfloat16
fp8 = mybir.dt.float8e4
AF = mybir.ActivationFunctionType
ALU = mybir.AluOpType
BF = ml_dtypes.bfloat16
F8 = ml_dtypes.float8_e4m3


def _bcast_rows(nc, pool, row_ap, nrows, ncols, tag, bufs=2, dtype=f32):
    """SBUF [nrows, ncols] tile = row_ap ([1, ncols] SBUF) broadcast
    across partitions, on the otherwise-idle GpSimd engine."""
    out = pool.tile([nrows, ncols], dtype, tag=tag, bufs=bufs, name=tag)
    nc.gpsimd.partition_broadcast(out, row_ap)
    return out


def build_program():
    # Bacc (not plain Bass): its compile() pass legalizes multi-semaphore
    # waits (event semaphores, nop fusion) that walrus can't encode raw.
    nc = bacc.Bacc()

    def _in(name, shape, dtype):
        return nc.declare_dram_parameter(name, shape, dtype, False)[:]

    xfm = _in("xfm", [D, L], f32)
    temb = _in("temb", [128, DC], f32)
    # wqk: host-packed so q-chunk hc and k-chunk hc are adjacent columns
    wqk = _in("wqk", [D, 2 * D], bf16)
    wv = _in("wv", [D, D], bf16)
    bq = _in("bq", [128, DC], f32)     # pre-scaled by 1/8
    bk = _in("bk", [128, DC], f32)
    bv = _in("bv", [1, D], bf16)
    wproj = _in("wproj", [D, D], bf16)
    bproj = _in("bproj", [128, DC], f32)
    w1 = _in("w1", [D, MLPD], bf16)
    b1 = _in("b1", [128, MC], f32)
    w2 = _in("w2", [MLPD, D], bf16)
    b2 = _in("b2", [128, DC], f32)
    wt = _in("wt", [D, 6 * D], bf16)
    bt = _in("bt", [128, 48], f32)
    out = nc.declare_dram_parameter("out_fm", [D, LOWN], f32, True)[:]

    with tile.TileContext(nc) as tc:
        _emit_kernel(tc, xfm, temb, wqk, wv, bq, bk, bv, wproj, bproj, w1, b1,
                     w2, b2, wt, bt, out)
    nc.finalize()  # runs Bacc.compile(): reg alloc + sync legalization
    return nc


def _emit_kernel(tc, xfm, temb, wqk, wv, bq, bk, bv, wproj, bproj, w1, b1,
                 w2, b2, wt, bt, out):
    nc = tc.nc

    # ---- persistent constants / host-prepped vectors (freed last) ----
    ones_f32, fr_ones_f32 = tc.tile([128, 1], f32, name="ones_f32")
    nc.vector.memset(ones_f32, 1.0)
    ones_bf, fr_ones_bf = tc.tile([128, 1], bf16, name="ones_bf")
    nc.vector.memset(ones_bf, 1.0)
    eps_tile, fr_eps = tc.tile([1, 1], f32, name="eps_tile")
    nc.vector.memset(eps_tile, 1e-5)

    bias_sb = {}
    bias_frees = []
    for name, ap, w in (("bq", bq, DC), ("bk", bk, DC), ("bproj", bproj, DC),
                        ("b1", b1, MC), ("b2", b2, DC), ("bt", bt, 48),
                        ("temb", temb, DC)):
        t, fr = tc.tile([128, w], f32, name=f"sb_{name}")
        nc.sync.dma_start(out=t, in_=ap)
        bias_sb[name] = t
        bias_frees.append(fr)
    # modulation vectors (computed in phase 0, consumed later)
    tp, fr_tp = tc.tile([128, 48], f32, name="tp")
    s_msa, fr_s1 = tc.tile([128, DC], f32, name="s_msa")
    s_mlp, fr_s2 = tc.tile([128, DC], f32, name="s_mlp")
    gmbp, fr_g1 = tc.tile([128, DC], f32, name="gmbp")
    gmb2, fr_g2 = tc.tile([128, DC], f32, name="gmb2")
    shift_msa = tp[:, 0:8]
    gate_msa = tp[:, 16:24]
    shift_mlp = tp[:, 24:32]
    gate_mlp = tp[:, 40:48]

    # ---- big persistent activations, creation order = reverse free order ----
    x_own, fr_x_own = tc.tile([128, DC, LOWN], f32, name="x_own")
    # v_aug: [token-part, token-chunk, head, 65]; col 64 holds ones so the
    # AV matmul also produces the softmax denominator.
    v_aug, fr_v = tc.tile([128, L // 128, H, HD + 1], bf16, name="v_aug")
    xmod, fr_xmod = tc.tile([128, DC, L], bf16, name="xmod")

    xr = xfm.rearrange("(c p) t -> p c t", p=128)
    nc.sync.dma_start(out=x_own, in_=xr[:, :, :LOWN])

    # ================= phase 0: time modulation vector ================
    with ExitStack() as ph:
        sbp = ph.enter_context(tc.tile_pool(name="p0_sb", bufs=2))
        psp = ph.enter_context(tc.tile_pool(name="p0_ps", bufs=1, space="PSUM"))
        sig = sbp.tile([128, DC], f32, tag="sig", bufs=1, name="sig")
        nc.scalar.activation(sig, bias_sb["temb"], AF.Sigmoid)
        silu_bf = sbp.tile([128, DC], bf16, tag="silu", bufs=1, name="silu_bf")
        nc.vector.tensor_tensor(silu_bf, bias_sb["temb"], sig, ALU.mult)

        # tp = silu @ Wt, weight-stationary; Wt streamed in 8 row-contiguous
        # [128, 6144] DMAs (12 KiB lines).
        ps_tp = psp.tile([128, 48], f32, name="ps_tp")
        for dc in range(DC):
            wt_sb = sbp.tile([128, 6 * D], bf16, tag="wt", bufs=2,
                             name="wt_sb")
            nc.sync.dma_start(out=wt_sb,
                              in_=wt[dc * 128:(dc + 1) * 128, :])
            for f in range(48):
                # start=True clears has_written for the WHOLE bank, so only
                # the very first matmul of this bank may carry it.
                nc.tensor.matmul(ps_tp[:, f:f + 1],
                                 wt_sb[:, f * 128:(f + 1) * 128],
                                 silu_bf[:, dc:dc + 1],
                                 start=(dc == 0 and f == 0),
                                 stop=(dc == DC - 1))
        nc.vector.tensor_tensor(tp, ps_tp, bias_sb["bt"], ALU.add)
        nc.vector.tensor_scalar_add(s_msa, tp[:, 8:16], 1.0)
        nc.vector.tensor_scalar_add(s_mlp, tp[:, 32:40], 1.0)
        nc.vector.tensor_tensor(gmbp, gate_msa, bias_sb["bproj"], ALU.mult)
        nc.vector.tensor_tensor(gmb2, gate_mlp, bias_sb["b2"], ALU.mult)

    # ---- LayerNorm-with-modulation helper (stats + apply for one l-tile) ----
    def ln_tile(sbp, psp, x_view, out_view, scale_ap, shift_ap, bc_bufs=2):
        """x_view: [128, DC, LT] f32; out = ((x-mu)*rstd)*s_d + sh_d.

        Stats: mean from f32 x; square-sums from a bf16 copy (made on the
        otherwise-idle scalar engine) so the big elementwise ops run in the
        DVE 2x bf16 mode."""
        xb = sbp.tile([128, DC, LT], bf16, tag="ln_xb", bufs=bc_bufs,
                      name="ln_xb")
        ps_s = psp.tile([1, LT], f32, tag="st_s", bufs=2, name="ps_s")
        ps_q = psp.tile([1, LT], f32, tag="st_q", bufs=2, name="ps_q")
        for dc in range(DC):
            xs = x_view[:, dc, :]
            nc.scalar.copy(xb[:, dc, :], xs)
            nc.tensor.matmul(ps_s, ones_f32, xs,
                             start=(dc == 0), stop=(dc == DC - 1))
            sq = sbp.tile([128, LT], bf16, tag="sq", bufs=bc_bufs, name="sq")
            nc.vector.tensor_tensor(sq, xb[:, dc, :], xb[:, dc, :], ALU.mult)
            nc.tensor.matmul(ps_q, ones_bf, sq,
                             start=(dc == 0), stop=(dc == DC - 1))
        mean = sbp.tile([1, LT], f32, tag="ln_mean", bufs=1, name="mean")
        var = sbp.tile([1, LT], f32, tag="ln_var", bufs=1, name="var")
        msq = sbp.tile([1, LT], f32, tag="ln_msq", bufs=1, name="msq")
        nc.vector.tensor_scalar_mul(mean, ps_s, 1.0 / D)
        nc.vector.tensor_scalar_mul(var, ps_q, 1.0 / D)
        nc.vector.tensor_tensor(msq, mean, mean, ALU.mult)
        nc.vector.tensor_tensor(var, var, msq, ALU.subtract)
        rstd = sbp.tile([1, LT], f32, tag="ln_rstd", bufs=1, name="rstd")
        nc.scalar.activation(rstd, var, AF.Sqrt, bias=eps_tile, scale=1.0)
        nc.vector.reciprocal(out=rstd, in_=rstd)
        a_bc = _bcast_rows(nc, sbp, rstd, 128, LT, "a_bc", bufs=bc_bufs)
        m_bc = _bcast_rows(nc, sbp, mean, 128, LT, "m_bc", bufs=bc_bufs)
        for dc in range(DC):
            # (x - mu) * rstd on the DVE; the per-feature (*s + sh) affine
            # rides the scalar engine's free scale/bias slots.
            t = sbp.tile([128, LT], bf16, tag="ln_t", bufs=2, name="ln_t")
            nc.vector.tensor_tensor(t, xb[:, dc, :], m_bc, ALU.subtract)
            nc.vector.tensor_tensor(t, t, a_bc, ALU.mult)
            nc.scalar.activation(out_view[:, dc, :], t, AF.Identity,
                                 bias=shift_ap[:, dc:dc + 1],
                                 scale=scale_ap[:, dc:dc + 1])

    # ================= phase 1: LN1 + modulate ================
    with ExitStack() as ph:
        sbp = ph.enter_context(tc.tile_pool(name="p1_sb", bufs=2))
        psp = ph.enter_context(tc.tile_pool(name="p1_ps", bufs=1, space="PSUM"))
        for lt in range(NLT_OWN):
            ln_tile(sbp, psp, x_own[:, :, lt * LT:(lt + 1) * LT],
                    xmod[:, :, lt * LT:(lt + 1) * LT], s_msa, shift_msa)
        # other token half is streamed, never fully resident
        for lt in range(NLT_OWN):
            xo = sbp.tile([128, DC, LT], f32, tag="xoth", bufs=2, name="xo")
            nc.sync.dma_start(
                out=xo, in_=xr[:, :, LOWN + lt * LT:LOWN + (lt + 1) * LT])
            ln_tile(sbp, psp, xo,
                    xmod[:, :, LOWN + lt * LT:LOWN + (lt + 1) * LT],
                    s_msa, shift_msa)

    # V inputs; V itself runs inside the pipeline as filler units so the
    # first softmax exp is gated only by the OWN-half LayerNorm.
    nc.vector.memset(v_aug[:, :, :, HD:], 1.0)
    bv_bc, fr_bv = tc.tile([128, D], bf16, name="bv_bc")
    nc.sync.dma_start(
        out=bv_bc,
        in_=bass.AP(tensor=bv.tensor, offset=bv.offset,
                    ap=[[0, 128]] + [list(x) for x in bv.ap[1:]]))
    wv_sb, fr_wv = tc.tile([128, DC, D], bf16, name="wv_sb")
    for dc in range(DC):
        nc.sync.dma_start(out=wv_sb[:, dc, :],
                          in_=wv[dc * 128:(dc + 1) * 128, :])

    # ============ phase 2b/3: fused QK projection + attention ============
    attn_sb, fr_attn = tc.tile([128, DC, LOWN], bf16, name="attn_sb")
    with ExitStack() as ph:
        sbp = ph.enter_context(tc.tile_pool(name="p23_sb", bufs=2))
        psp = ph.enter_context(tc.tile_pool(name="p23_ps", bufs=1,
                                            space="PSUM"))

        def load_wqk(hc):
            w = sbp.tile([128, DC, 256], bf16, tag="wqk", bufs=2, name="wqk")
            for dc in range(DC):
                nc.sync.dma_start(
                    out=w[:, dc, :],
                    in_=wqk[dc * 128:(dc + 1) * 128, hc * 256:(hc + 1) * 256])
            return w

        def qkproj_units(hc, w, bias_eng="vector"):
            """Allocate this chunk's q/k tiles; return emission closures,
            one per output token-tile (2 q + 4 k).  bias_eng="scalar" is used
            for head-chunk 0 so its biases don't queue behind the other
            half's LayerNorm work on the vector engine."""
            qt = sbp.tile([128, LOWN], bf16, tag="qch", bufs=2, name="qch")
            kt = sbp.tile([128, L], bf16, tag="kch", bufs=2, name="kch")

            def mk(kind, lt):
                def emit():
                    ps = psp.tile([128, LT], f32, tag="qk", bufs=2,
                                  name="ps_qk")
                    off = kind * 128
                    for dc in range(DC):
                        nc.tensor.matmul(
                            ps, w[:, dc, off:off + 128],
                            xmod[:, dc, lt * LT:(lt + 1) * LT],
                            start=(dc == 0), stop=(dc == DC - 1))
                    if kind == 0:
                        dst = qt[:, lt * LT:(lt + 1) * LT]
                        if bias_eng == "scalar":
                            nc.scalar.activation(
                                dst, ps, AF.Identity,
                                bias=bias_sb["bq"][:, hc:hc + 1], scale=0.125)
                        else:
                            nc.vector.tensor_scalar(
                                out=dst, in0=ps, scalar1=0.125,
                                scalar2=bias_sb["bq"][:, hc:hc + 1],
                                op0=ALU.mult, op1=ALU.add)
                    else:
                        dst = kt[:, lt * LT:(lt + 1) * LT]
                        if bias_eng == "scalar":
                            nc.scalar.activation(
                                dst, ps, AF.Identity,
                                bias=bias_sb["bk"][:, hc:hc + 1])
                        else:
                            nc.vector.tensor_scalar_add(
                                dst, ps, bias_sb["bk"][:, hc:hc + 1])
                return emit

            units = [mk(0, lt) for lt in range(NLT_OWN)]
            units += [mk(1, lt) for lt in range(L // LT)]
            return qt, kt, units

        def finish_head(hc, lt, i, ps_av):
            lts = slice(lt * LT, (lt + 1) * LT)
            rcp = sbp.tile([1, LT], f32, tag="rcp", bufs=2, name="rcp")
            nc.vector.reciprocal(out=rcp, in_=ps_av[HD:HD + 1, :])
            rcp_bc = _bcast_rows(nc, sbp, rcp, 64, LT, "rcp_bc", bufs=1)
            if i == 0:
                # partitions line up: write attention output in place
                nc.vector.tensor_tensor(attn_sb[0:64, hc, lts],
                                        ps_av[:HD, :], rcp_bc, ALU.mult)
            else:
                at = sbp.tile([64, LT], bf16, tag="at", bufs=2, name="at")
                nc.vector.tensor_tensor(at, ps_av[:HD, :], rcp_bc, ALU.mult)
                nc.sync.dma_start(out=attn_sb[64:128, hc, lts], in_=at)

        def v_units():
            """V projection (x-stationary, token-major) as 8 filler units of
            2 token-chunks each; psum rides the av tag (idle during the
            first head-chunk's lt0 stretches)."""
            def mk(tc0):
                def emit():
                    for tcn in (tc0, tc0 + 1):
                        for vs in range(2):
                            psv = psp.tile([128, LT], f32, tag="av", bufs=2,
                                           name="ps_v")
                            for dc in range(DC):
                                nc.tensor.matmul(
                                    psv,
                                    xmod[:, dc, tcn * 128:(tcn + 1) * 128],
                                    wv_sb[:, dc, vs * LT:(vs + 1) * LT],
                                    start=(dc == 0), stop=(dc == DC - 1))
                            nc.vector.tensor_tensor(
                                v_aug[:, tcn, vs * 8:(vs + 1) * 8, :HD],
                                psv, bv_bc[:, vs * LT:(vs + 1) * LT], ALU.add)
                return emit
            return [mk(t) for t in range(0, L // 128, 2)]

        def av_units(hc, lt, eh_lt):
            """4 closures: (head, half) AV accumulations; half 1 finishes."""
            state = {}

            def mk(i, half):
                def emit():
                    if half == 0:
                        state[i] = psp.tile([HD + 1, LT], f32, tag="av",
                                            bufs=2, name="ps_av")
                    ps_av = state[i]
                    for mcn in range(half * 8, half * 8 + 8):
                        nc.tensor.matmul(
                            ps_av, v_aug[:, mcn, 2 * hc + i, :],
                            eh_lt[i][half][:, mcn - half * 8, :],
                            start=(mcn == 0), stop=(mcn == L // 128 - 1))
                    if half == 1:
                        finish_head(hc, lt, i, ps_av)
                return emit

            return [mk(i, half) for i in range(2) for half in range(2)]

        def scores_stretch(qt, kt, eh_lt, lt, mg):
            lts = slice(lt * LT, (lt + 1) * LT)
            if mg % 4 == 0:
                for i in range(2):
                    eh_lt[i][mg // 4] = sbp.tile(
                        [128, 8, LT], bf16, tag=f"ept{i}", bufs=3,
                        name=f"ept{i}")
            ps_pair = [psp.tile([128, 2, LT], f32, tag="sc", bufs=2,
                                name="ps_sc") for _ in range(2)]
            for j in range(2):
                ms = slice((mg * 2 + j) * 128, (mg * 2 + j + 1) * 128)
                nc.tensor.matmul(ps_pair[0][:, j, :], kt[0:64, ms],
                                 qt[0:64, lts],
                                 start=True, stop=True, tile_position=(0, 0))
                nc.tensor.matmul(ps_pair[1][:, j, :], kt[64:128, ms],
                                 qt[64:128, lts],
                                 start=True, stop=True, tile_position=(64, 0))
            for i in range(2):
                nc.scalar.activation(
                    eh_lt[i][mg // 4][:, (mg % 4) * 2:(mg % 4) * 2 + 2, :],
                    ps_pair[i], AF.Exp)

        # prologue: only q + k token-tiles 0/1 (gated by the OWN-half LN)
        # run before the first scores; k2/k3 (other half) and all of V ride
        # the first head-chunk's stretches as fillers.
        w0 = load_wqk(0)
        qt, kt, units0 = qkproj_units(0, w0, bias_eng="scalar")
        for u in units0[:4]:
            u()
        prologue_rest = units0[4:]  # k2, k3
        cur = (qt, kt)
        pend_av = []  # AV units of (hc-1, lt1), scheduled into lt0 stretches

        for hc in range(NHC):
            if hc + 1 < NHC:
                w_n = load_wqk(hc + 1)
                qt_n, kt_n, units_n = qkproj_units(hc + 1, w_n)
            else:
                units_n = []
            qt, kt = cur
            eh = [[[None, None] for _ in range(2)] for _ in range(2)]
            for lt in range(2):
                if lt == 0:
                    fill = deque(pend_av)
                    pend_av = []
                    if hc == 0:
                        fill.extend(prologue_rest)  # k2, k3
                        fill.extend(v_units())      # all 16 V token-chunks
                        fill.extend(units_n[:2])
                    else:
                        fill.extend(units_n[:3])
                else:
                    fill = deque(av_units(hc, 0, eh[0]))
                    fill.extend(units_n[2:] if hc == 0 else units_n[3:])
                for mg in range(8):
                    scores_stretch(qt, kt, eh[lt], lt, mg)
                    if fill:
                        fill.popleft()()
                    if mg == 7:
                        while fill:
                            fill.popleft()()
            pend_av = av_units(hc, 1, eh[1])
            if hc + 1 < NHC:
                cur = (qt_n, kt_n)
        for u in pend_av:
            u()

    # ================= phase 4: proj + residual ================
    with ExitStack() as ph:
        sbp = ph.enter_context(tc.tile_pool(name="p4_sb", bufs=2))
        psp = ph.enter_context(tc.tile_pool(name="p4_ps", bufs=1, space="PSUM"))
        wpbig = sbp.tile([128, DC, D], bf16, tag="wp", bufs=1, name="wpbig")
        for dc in range(DC):
            nc.sync.dma_start(out=wpbig[:, dc, :],
                              in_=wproj[dc * 128:(dc + 1) * 128, :])
        for lt in range(NLT_OWN):
            for ft in range(DC):
                ps = psp.tile([128, LT], f32, tag="pj", bufs=4, name="ps_pj")
                for dc in range(DC):
                    nc.tensor.matmul(
                        ps, wpbig[:, dc, ft * 128:(ft + 1) * 128],
                        attn_sb[:, dc, lt * LT:(lt + 1) * LT],
                        start=(dc == 0), stop=(dc == DC - 1))
                gh = sbp.tile([128, LT], f32, tag="gh", bufs=3, name="gh")
                nc.scalar.activation(gh, ps, AF.Identity,
                                     bias=gmbp[:, ft:ft + 1],
                                     scale=gate_msa[:, ft:ft + 1])
                xo = x_own[:, ft, lt * LT:(lt + 1) * LT]
                nc.vector.tensor_tensor(xo, xo, gh, ALU.add)
    fr_attn()
    fr_wv()
    fr_bv()
    fr_xmod()
    fr_v()

    # ================= phase 5/6: LN2 + MLP ================
    outr = out.rearrange("(c p) t -> p c t", p=128)
    gelu_sb, fr_gelu = tc.tile([128, MC, LOWN], bf16, name="gelu_sb")
    with ExitStack() as ph:
        sbp = ph.enter_context(tc.tile_pool(name="p5_sb", bufs=2))
        psp = ph.enter_context(tc.tile_pool(name="p5_ps", bufs=1, space="PSUM"))
        h2mod = sbp.tile([128, DC, LOWN], bf16, tag="h2", bufs=1, name="h2mod")
        w1big = sbp.tile([128, DC, MLPD], bf16, tag="w1", bufs=1, name="w1big")
        for dc in range(DC):
            nc.sync.dma_start(out=w1big[:, dc, :],
                              in_=w1[dc * 128:(dc + 1) * 128, :])
        for lt in range(NLT_OWN):
            ln_tile(sbp, psp, x_own[:, :, lt * LT:(lt + 1) * LT],
                    h2mod[:, :, lt * LT:(lt + 1) * LT], s_mlp, shift_mlp,
                    bc_bufs=1)
        for ft in range(MC):
            # one [128, 2, 512] psum tile per ft: both token tiles, and
            # a single batched gelu over 1024 columns
            ps = psp.tile([128, NLT_OWN, LT], f32, tag="f1", bufs=2,
                          name="ps_f1")
            for dc in range(DC):
                wti = w1big[:, dc, ft * 128:(ft + 1) * 128]
                for lt in range(NLT_OWN):
                    nc.tensor.matmul(
                        ps[:, lt, :], wti,
                        h2mod[:, dc, lt * LT:(lt + 1) * LT],
                        start=(dc == 0), stop=(dc == DC - 1))
            nc.scalar.activation(gelu_sb[:, ft, :], ps, AF.Gelu,
                                 bias=bias_sb["b1"][:, ft:ft + 1])

    # fc2: ft-outer with resident w2 so each stationary serves both token
    # tiles (LDWEIGHTS amortized); fc1's pool is closed to make room.
    with ExitStack() as ph:
        sbp = ph.enter_context(tc.tile_pool(name="p7_sb", bufs=2))
        psp2 = ph.enter_context(tc.tile_pool(name="p7_ps", bufs=1,
                                             space="PSUM"))
        w2big = sbp.tile([128, MC, D], bf16, tag="w2", bufs=1, name="w2big")
        for mc in range(MC):
            nc.sync.dma_start(out=w2big[:, mc, :],
                              in_=w2[mc * 128:(mc + 1) * 128, :])
        for ft in range(DC):
            ps = [psp2.tile([128, LT], f32, tag=f"f2{i}", bufs=2,
                            name=f"ps_f2{i}") for i in range(NLT_OWN)]
            for mc in range(MC):
                wti = w2big[:, mc, ft * 128:(ft + 1) * 128]
                for lt in range(NLT_OWN):
                    nc.tensor.matmul(
                        ps[lt], wti, gelu_sb[:, mc, lt * LT:(lt + 1) * LT],
                        start=(mc == 0), stop=(mc == MC - 1))
            for lt in range(NLT_OWN):
                gh = sbp.tile([128, LT], f32, tag="gh2", bufs=3, name="gh2")
                nc.scalar.activation(gh, ps[lt], AF.Identity,
                                     bias=gmb2[:, ft:ft + 1],
                                     scale=gate_mlp[:, ft:ft + 1])
                xo = x_own[:, ft, lt * LT:(lt + 1) * LT]
                nc.vector.tensor_tensor(xo, xo, gh, ALU.add)
            # stream this feature chunk of the output right away
            nc.sync.dma_start(out=outr[:, ft, :], in_=x_own[:, ft, :])
    fr_gelu()

    # (output halves are streamed out at the end of each fc2 token tile)

    # release persistents in reverse creation order
    fr_x_own()
    fr_g2(); fr_g1(); fr_s2(); fr_s1(); fr_tp()
    for fr in reversed(bias_frees):
        fr()
    fr_eps(); fr_ones_bf(); fr_ones_f32()


_PROGRAM_CACHE = {}


def _get_program():
    if "nc" not in _PROGRAM_CACHE:
        _PROGRAM_CACHE["nc"] = build_program()
    return _PROGRAM_CACHE["nc"]


def _fm(v):
    """[D] vector -> feature-major [128, D//128] (partition p, chunk c)."""
    return np.ascontiguousarray(np.asarray(v, np.float32).reshape(-1, 128).T)


def make_in_maps(x, time_emb, Wqkv, bqkv, Wproj, bproj, W1, b1, W2, b2, Wt, bt,
                 g1, be1, g2, be2):
    # g1/be1/g2/be2 are identity layernorm params in this module; verify and
    # fold them away.
    assert np.allclose(g1, 1.0) and np.allclose(g2, 1.0)
    assert np.allclose(be1, 0.0) and np.allclose(be2, 0.0)

    x = np.asarray(x, np.float32)
    wqkv_f = np.asarray(Wqkv, np.float32)
    # interleave q-chunk hc / k-chunk hc column blocks of 128
    wqk_packed = np.empty((D, 2 * D), np.float32)
    for hc in range(DC):
        wqk_packed[:, hc * 256:hc * 256 + 128] = \
            wqkv_f[:, hc * 128:(hc + 1) * 128]
        wqk_packed[:, hc * 256 + 128:(hc + 1) * 256] = \
            wqkv_f[:, D + hc * 128:D + (hc + 1) * 128]
    shared = {
        "wqk": wqk_packed.astype(BF),
        "wv": np.ascontiguousarray(wqkv_f[:, 2 * D:]).astype(BF),
        "bq": _fm(np.asarray(bqkv[:D]) * 0.125),
        "bk": _fm(bqkv[D:2 * D]),
        "bv": np.ascontiguousarray(
            np.asarray(bqkv[2 * D:], np.float32)[None, :]).astype(BF),
        "wproj": np.asarray(Wproj, np.float32).astype(BF),
        "bproj": _fm(bproj),
        "w1": np.asarray(W1, np.float32).astype(BF),
        "b1": _fm(b1),
        "w2": np.asarray(W2, np.float32).astype(BF),
        "b2": _fm(b2),
        "wt": np.asarray(Wt, np.float32).astype(BF),
        "bt": _fm(bt),
    }
    in_maps = []
    for c in range(NCORES):
        b, half = c // 2, c % 2
        xb = x[b].T  # [D, L] feature-major
        own = slice(half * LOWN, (half + 1) * LOWN)
        oth = slice((1 - half) * LOWN, (2 - half) * LOWN)
        m = dict(shared)
        m["xfm"] = np.ascontiguousarray(
            np.concatenate([xb[:, own], xb[:, oth]], axis=1))
        m["temb"] = _fm(time_emb[b])
        in_maps.append(m)
    return in_maps


def assemble_output(results):
    outp = np.empty((B, L, D), np.float32)
    for c in range(NCORES):
        b, half = c // 2, c % 2
        outp[b, half * LOWN:(half + 1) * LOWN, :] = results[c]["out_fm"].T
    return outp


def kernel(x, time_emb, Wqkv, bqkv, Wproj, bproj, W1, b1, W2, b2, Wt, bt,
           g1, be1, g2, be2, trace=False, trace_kwargs=None, trace_cores=None):
    in_maps = make_in_maps(x, time_emb, Wqkv, bqkv, Wproj, bproj, W1, b1,
                           W2, b2, Wt, bt, g1, be1, g2, be2)
    nc = _get_program()
    res = run_bass_kernel_spmd(nc, in_maps, core_ids=list(range(NCORES)),
                               trace=trace, trace_kwargs=trace_kwargs or {},
                               trace_cores=trace_cores)
    kernel.last_results = res
    return assemble_output(res.results)


# revision 55
# speedup vs baseline: 1.1806x; 1.1806x over previous
"""DiT block (adaLN) Trainium2 kernel, 8-core SPMD, no collectives.

Sharding: core c handles batch b = c//2 and query-token half c%2 (1024 q
tokens).  Each core computes K/V for all 2048 tokens of its batch (the
only duplicated work), so cores never communicate.  The host permutes
each core's token columns so its own 1024 tokens come first (softmax is
invariant to key order), and transposes x to feature-major [D, L] so the
device never transposes anything.

On-device layout is feature-major everywhere: activations live as
[128 partitions, d-chunk, tokens].  LayerNorm stats (per-token = free
dim) are computed with ones-vector matmuls on the tensor engine and
broadcast back across partitions on GpSimd.  All GEMM operands are bf16
(fp32 PSUM accumulation); the residual stream, softmax and LN statistics
stay fp32.

The QKV projection and attention are fused into one software-pipelined
stream over head-chunks: softmax exp saturates the scalar engine
(~37us/chunk), so the in-order tensor engine is kept busy by hand-
interleaving independent work (next chunk's q/k projections, previous
tile's AV matmuls) between the exp-gated score matmuls.  Weights are
DMA'd in large row-contiguous tiles and sliced on-chip.
"""

import os
import sys
from collections import deque
from contextlib import ExitStack

os.environ.setdefault("MYCRO_LOCAL_CACHE", "1")
for _p in ("/opt/trn_rl_repo", "/root/.axon_site/_ro/trn_rl_repo"):
    if os.path.isdir(_p) and _p not in sys.path:
        sys.path.insert(0, _p)

import ml_dtypes
import numpy as np

import concourse.bass as bass
import concourse.tile as tile
from concourse import bacc, mybir
from concourse.bass_utils import run_bass_kernel_spmd

B, L, D, H, HD, MLPD = 4, 2048, 1024, 16, 64, 4096
NCORES = 8
LOWN = L // 2          # own query tokens per core
DC = D // 128          # 8 chunks of the model dim
MC = MLPD // 128       # 32 chunks of the mlp dim
LT = 512               # token tile for matmul free dim
NLT_OWN = LOWN // LT   # 2 token tiles (queries)
NHC = H // 2           # head-pair chunks

f32 = mybir.dt.float32
bf16 = mybir.dt.bfloat16
fp8 = mybir.dt.float8e4
AF = mybir.ActivationFunctionType
ALU = mybir.AluOpType
BF = ml_dtypes.bfloat16
F8 = ml_dtypes.float8_e4m3


def _bcast_rows(nc, pool, row_ap, nrows, ncols, tag, bufs=2, dtype=f32):
    """SBUF [nrows, ncols] tile = row_ap ([1, ncols] SBUF) broadcast
    across partitions, on the otherwise-idle GpSimd engine."""
    out = pool.tile([nrows, ncols], dtype, tag=tag, bufs=bufs, name=tag)
    nc.gpsimd.partition_broadcast(out, row_ap)
    return out


def build_program():
    # Bacc (not plain Bass): its compile() pass legalizes multi-semaphore
    # waits (event semaphores, nop fusion) that walrus can't encode raw.
    nc = bacc.Bacc()

    def _in(name, shape, dtype):
        return nc.declare_dram_parameter(name, shape, dtype, False)[:]

    xfm = _in("xfm", [D, L], f32)
    temb = _in("temb", [128, DC], f32)
    # wqk: host-packed so q-chunk hc and k-chunk hc are adjacent columns
    wqk = _in("wqk", [D, 2 * D], bf16)
    wv = _in("wv", [D, D], bf16)
    bq = _in("bq", [128, DC], f32)     # pre-scaled by 1/8
    bk = _in("bk", [128, DC], f32)
    bv = _in("bv", [1, D], bf16)
    wproj = _in("wproj", [D, D], bf16)
    bproj = _in("bproj", [128, DC], f32)
    w1 = _in("w1", [D, MLPD], bf16)
    b1 = _in("b1", [128, MC], f32)
    w2 = _in("w2", [MLPD, D], bf16)
    b2 = _in("b2", [128, DC], f32)
    wt = _in("wt", [D, 6 * D], bf16)
    bt = _in("bt", [128, 48], f32)
    out = nc.declare_dram_parameter("out_fm", [D, LOWN], f32, True)[:]

    with tile.TileContext(nc) as tc:
        _emit_kernel(tc, xfm, temb, wqk, wv, bq, bk, bv, wproj, bproj, w1, b1,
                     w2, b2, wt, bt, out)
    nc.finalize()  # runs Bacc.compile(): reg alloc + sync legalization
    return nc


def _emit_kernel(tc, xfm, temb, wqk, wv, bq, bk, bv, wproj, bproj, w1, b1,
                 w2, b2, wt, bt, out):
    nc = tc.nc

    # ---- persistent constants / host-prepped vectors (freed last) ----
    ones_f32, fr_ones_f32 = tc.tile([128, 1], f32, name="ones_f32")
    nc.vector.memset(ones_f32, 1.0)
    ones_bf, fr_ones_bf = tc.tile([128, 1], bf16, name="ones_bf")
    nc.vector.memset(ones_bf, 1.0)
    eps_tile, fr_eps = tc.tile([1, 1], f32, name="eps_tile")
    nc.vector.memset(eps_tile, 1e-5)

    bias_sb = {}
    bias_frees = []
    for name, ap, w in (("bq", bq, DC), ("bk", bk, DC), ("bproj", bproj, DC),
                        ("b1", b1, MC), ("b2", b2, DC), ("bt", bt, 48),
                        ("temb", temb, DC)):
        t, fr = tc.tile([128, w], f32, name=f"sb_{name}")
        nc.sync.dma_start(out=t, in_=ap)
        bias_sb[name] = t
        bias_frees.append(fr)
    # modulation vectors (computed in phase 0, consumed later)
    tp, fr_tp = tc.tile([128, 48], f32, name="tp")
    s_msa, fr_s1 = tc.tile([128, DC], f32, name="s_msa")
    s_mlp, fr_s2 = tc.tile([128, DC], f32, name="s_mlp")
    gmbp, fr_g1 = tc.tile([128, DC], f32, name="gmbp")
    gmb2, fr_g2 = tc.tile([128, DC], f32, name="gmb2")
    shift_msa = tp[:, 0:8]
    gate_msa = tp[:, 16:24]
    shift_mlp = tp[:, 24:32]
    gate_mlp = tp[:, 40:48]

    # ---- big persistent activations, creation order = reverse free order ----
    x_own, fr_x_own = tc.tile([128, DC, LOWN], f32, name="x_own")
    # v_aug: [token-part, token-chunk, head, 65]; col 64 holds ones so the
    # AV matmul also produces the softmax denominator.
    v_aug, fr_v = tc.tile([128, L // 128, H, HD + 1], bf16, name="v_aug")
    xmod, fr_xmod = tc.tile([128, DC, L], bf16, name="xmod")

    xr = xfm.rearrange("(c p) t -> p c t", p=128)
    # split per token-tile so LN1 stats start when the first half lands
    for lt in range(NLT_OWN):
        nc.sync.dma_start(out=x_own[:, :, lt * LT:(lt + 1) * LT],
                          in_=xr[:, :, lt * LT:(lt + 1) * LT])

    # ================= phase 0: time modulation vector ================
    with ExitStack() as ph:
        sbp = ph.enter_context(tc.tile_pool(name="p0_sb", bufs=2))
        psp = ph.enter_context(tc.tile_pool(name="p0_ps", bufs=1, space="PSUM"))
        sig = sbp.tile([128, DC], f32, tag="sig", bufs=1, name="sig")
        nc.scalar.activation(sig, bias_sb["temb"], AF.Sigmoid)
        silu_bf = sbp.tile([128, DC], bf16, tag="silu", bufs=1, name="silu_bf")
        nc.vector.tensor_tensor(silu_bf, bias_sb["temb"], sig, ALU.mult)

        # tp = silu @ Wt, weight-stationary; Wt streamed in 8 row-contiguous
        # [128, 6144] DMAs (12 KiB lines).
        ps_tp = psp.tile([128, 48], f32, name="ps_tp")
        for dc in range(DC):
            wt_sb = sbp.tile([128, 6 * D], bf16, tag="wt", bufs=2,
                             name="wt_sb")
            nc.sync.dma_start(out=wt_sb,
                              in_=wt[dc * 128:(dc + 1) * 128, :])
            for f in range(48):
                # start=True clears has_written for the WHOLE bank, so only
                # the very first matmul of this bank may carry it.
                nc.tensor.matmul(ps_tp[:, f:f + 1],
                                 wt_sb[:, f * 128:(f + 1) * 128],
                                 silu_bf[:, dc:dc + 1],
                                 start=(dc == 0 and f == 0),
                                 stop=(dc == DC - 1))
        nc.vector.tensor_tensor(tp, ps_tp, bias_sb["bt"], ALU.add)
        nc.vector.tensor_scalar_add(s_msa, tp[:, 8:16], 1.0)
        nc.vector.tensor_scalar_add(s_mlp, tp[:, 32:40], 1.0)
        nc.vector.tensor_tensor(gmbp, gate_msa, bias_sb["bproj"], ALU.mult)
        nc.vector.tensor_tensor(gmb2, gate_mlp, bias_sb["b2"], ALU.mult)

    # ---- LayerNorm-with-modulation helper (stats + apply for one l-tile) ----
    def ln_tile(sbp, psp, x_view, out_view, scale_ap, shift_ap, bc_bufs=2):
        """x_view: [128, DC, LT] f32; out = ((x-mu)*rstd)*s_d + sh_d.

        Stats: mean from f32 x; square-sums from a bf16 copy (made on the
        otherwise-idle scalar engine) so the big elementwise ops run in the
        DVE 2x bf16 mode."""
        xb = sbp.tile([128, DC, LT], bf16, tag="ln_xb", bufs=bc_bufs,
                      name="ln_xb")
        ps_s = psp.tile([1, LT], f32, tag="st_s", bufs=2, name="ps_s")
        ps_q = psp.tile([1, LT], f32, tag="st_q", bufs=2, name="ps_q")
        for dc in range(DC):
            xs = x_view[:, dc, :]
            nc.scalar.copy(xb[:, dc, :], xs)
            nc.tensor.matmul(ps_s, ones_f32, xs,
                             start=(dc == 0), stop=(dc == DC - 1))
            sq = sbp.tile([128, LT], bf16, tag="sq", bufs=bc_bufs, name="sq")
            nc.vector.tensor_tensor(sq, xb[:, dc, :], xb[:, dc, :], ALU.mult)
            nc.tensor.matmul(ps_q, ones_bf, sq,
                             start=(dc == 0), stop=(dc == DC - 1))
        mean = sbp.tile([1, LT], f32, tag="ln_mean", bufs=1, name="mean")
        var = sbp.tile([1, LT], f32, tag="ln_var", bufs=1, name="var")
        msq = sbp.tile([1, LT], f32, tag="ln_msq", bufs=1, name="msq")
        nc.vector.tensor_scalar_mul(mean, ps_s, 1.0 / D)
        nc.vector.tensor_scalar_mul(var, ps_q, 1.0 / D)
        nc.vector.tensor_tensor(msq, mean, mean, ALU.mult)
        nc.vector.tensor_tensor(var, var, msq, ALU.subtract)
        rstd = sbp.tile([1, LT], f32, tag="ln_rstd", bufs=1, name="rstd")
        nc.scalar.activation(rstd, var, AF.Sqrt, bias=eps_tile, scale=1.0)
        nc.vector.reciprocal(out=rstd, in_=rstd)
        a_bc = _bcast_rows(nc, sbp, rstd, 128, LT, "a_bc", bufs=bc_bufs)
        m_bc = _bcast_rows(nc, sbp, mean, 128, LT, "m_bc", bufs=bc_bufs)
        for dc in range(DC):
            # (x - mu) * rstd on the DVE; the per-feature (*s + sh) affine
            # rides the scalar engine's free scale/bias slots.
            t = sbp.tile([128, LT], bf16, tag="ln_t", bufs=2, name="ln_t")
            nc.vector.tensor_tensor(t, xb[:, dc, :], m_bc, ALU.subtract)
            nc.vector.tensor_tensor(t, t, a_bc, ALU.mult)
            nc.scalar.activation(out_view[:, dc, :], t, AF.Identity,
                                 bias=shift_ap[:, dc:dc + 1],
                                 scale=scale_ap[:, dc:dc + 1])

    # ================= phase 1: LN1 + modulate ================
    with ExitStack() as ph:
        sbp = ph.enter_context(tc.tile_pool(name="p1_sb", bufs=2))
        psp = ph.enter_context(tc.tile_pool(name="p1_ps", bufs=1, space="PSUM"))
        for lt in range(NLT_OWN):
            ln_tile(sbp, psp, x_own[:, :, lt * LT:(lt + 1) * LT],
                    xmod[:, :, lt * LT:(lt + 1) * LT], s_msa, shift_msa)
        # other token half is streamed, never fully resident
        for lt in range(NLT_OWN):
            xo = sbp.tile([128, DC, LT], f32, tag="xoth", bufs=2, name="xo")
            nc.sync.dma_start(
                out=xo, in_=xr[:, :, LOWN + lt * LT:LOWN + (lt + 1) * LT])
            ln_tile(sbp, psp, xo,
                    xmod[:, :, LOWN + lt * LT:LOWN + (lt + 1) * LT],
                    s_msa, shift_msa)

    # V inputs; V itself runs inside the pipeline as filler units so the
    # first softmax exp is gated only by the OWN-half LayerNorm.
    nc.vector.memset(v_aug[:, :, :, HD:], 1.0)
    bv_bc, fr_bv = tc.tile([128, D], bf16, name="bv_bc")
    nc.sync.dma_start(
        out=bv_bc,
        in_=bass.AP(tensor=bv.tensor, offset=bv.offset,
                    ap=[[0, 128]] + [list(x) for x in bv.ap[1:]]))
    wv_sb, fr_wv = tc.tile([128, DC, D], bf16, name="wv_sb")
    for dc in range(DC):
        nc.sync.dma_start(out=wv_sb[:, dc, :],
                          in_=wv[dc * 128:(dc + 1) * 128, :])

    # ============ phase 2b/3: fused QK projection + attention ============
    attn_sb, fr_attn = tc.tile([128, DC, LOWN], bf16, name="attn_sb")
    with ExitStack() as ph:
        sbp = ph.enter_context(tc.tile_pool(name="p23_sb", bufs=2))
        psp = ph.enter_context(tc.tile_pool(name="p23_ps", bufs=1,
                                            space="PSUM"))

        def load_wqk(hc):
            w = sbp.tile([128, DC, 256], bf16, tag="wqk", bufs=2, name="wqk")
            for dc in range(DC):
                nc.sync.dma_start(
                    out=w[:, dc, :],
                    in_=wqk[dc * 128:(dc + 1) * 128, hc * 256:(hc + 1) * 256])
            return w

        def qkproj_units(hc, w):
            """Allocate this chunk's q/k tiles; return emission closures,
            one per output token-tile (2 q + 4 k)."""
            qt = sbp.tile([128, LOWN], bf16, tag="qch", bufs=2, name="qch")
            kt = sbp.tile([128, L], bf16, tag="kch", bufs=2, name="kch")

            def mk(kind, lt):
                def emit():
                    ps = psp.tile([128, LT], f32, tag="qk", bufs=2,
                                  name="ps_qk")
                    off = kind * 128
                    for dc in range(DC):
                        nc.tensor.matmul(
                            ps, w[:, dc, off:off + 128],
                            xmod[:, dc, lt * LT:(lt + 1) * LT],
                            start=(dc == 0), stop=(dc == DC - 1))
                    if kind == 0:
                        nc.vector.tensor_scalar(
                            out=qt[:, lt * LT:(lt + 1) * LT], in0=ps,
                            scalar1=0.125,
                            scalar2=bias_sb["bq"][:, hc:hc + 1],
                            op0=ALU.mult, op1=ALU.add)
                    else:
                        nc.vector.tensor_scalar_add(
                            kt[:, lt * LT:(lt + 1) * LT], ps,
                            bias_sb["bk"][:, hc:hc + 1])
                return emit

            units = [mk(0, lt) for lt in range(NLT_OWN)]
            units += [mk(1, lt) for lt in range(L // LT)]
            return qt, kt, units

        def finish_head(hc, lt, i, ps_av):
            lts = slice(lt * LT, (lt + 1) * LT)
            rcp = sbp.tile([1, LT], f32, tag="rcp", bufs=2, name="rcp")
            nc.vector.reciprocal(out=rcp, in_=ps_av[HD:HD + 1, :])
            rcp_bc = _bcast_rows(nc, sbp, rcp, 64, LT, "rcp_bc", bufs=1)
            if i == 0:
                # partitions line up: write attention output in place
                nc.vector.tensor_tensor(attn_sb[0:64, hc, lts],
                                        ps_av[:HD, :], rcp_bc, ALU.mult)
            else:
                at = sbp.tile([64, LT], bf16, tag="at", bufs=2, name="at")
                nc.vector.tensor_tensor(at, ps_av[:HD, :], rcp_bc, ALU.mult)
                nc.sync.dma_start(out=attn_sb[64:128, hc, lts], in_=at)

        def v_units():
            """V projection (x-stationary, token-major) as 8 filler units of
            2 token-chunks each; psum rides the av tag (idle during the
            first head-chunk's lt0 stretches)."""
            def mk(tc0):
                def emit():
                    for tcn in (tc0, tc0 + 1):
                        for vs in range(2):
                            psv = psp.tile([128, LT], f32, tag="av", bufs=2,
                                           name="ps_v")
                            for dc in range(DC):
                                nc.tensor.matmul(
                                    psv,
                                    xmod[:, dc, tcn * 128:(tcn + 1) * 128],
                                    wv_sb[:, dc, vs * LT:(vs + 1) * LT],
                                    start=(dc == 0), stop=(dc == DC - 1))
                            nc.vector.tensor_tensor(
                                v_aug[:, tcn, vs * 8:(vs + 1) * 8, :HD],
                                psv, bv_bc[:, vs * LT:(vs + 1) * LT], ALU.add)
                return emit
            return [mk(t) for t in range(0, L // 128, 2)]

        def av_units(hc, lt, eh_lt):
            """4 closures: (head, half) AV accumulations; half 1 finishes."""
            state = {}

            def mk(i, half):
                def emit():
                    if half == 0:
                        state[i] = psp.tile([HD + 1, LT], f32, tag="av",
                                            bufs=2, name="ps_av")
                    ps_av = state[i]
                    for mcn in range(half * 8, half * 8 + 8):
                        nc.tensor.matmul(
                            ps_av, v_aug[:, mcn, 2 * hc + i, :],
                            eh_lt[i][half][:, mcn - half * 8, :],
                            start=(mcn == 0), stop=(mcn == L // 128 - 1))
                    if half == 1:
                        finish_head(hc, lt, i, ps_av)
                return emit

            return [mk(i, half) for i in range(2) for half in range(2)]

        def scores_stretch(qt, kt, eh_lt, lt, mg):
            lts = slice(lt * LT, (lt + 1) * LT)
            if mg % 4 == 0:
                for i in range(2):
                    eh_lt[i][mg // 4] = sbp.tile(
                        [128, 8, LT], bf16, tag=f"ept{i}", bufs=3,
                        name=f"ept{i}")
            ps_pair = [psp.tile([128, 2, LT], f32, tag="sc", bufs=2,
                                name="ps_sc") for _ in range(2)]
            for j in range(2):
                ms = slice((mg * 2 + j) * 128, (mg * 2 + j + 1) * 128)
                nc.tensor.matmul(ps_pair[0][:, j, :], kt[0:64, ms],
                                 qt[0:64, lts],
                                 start=True, stop=True, tile_position=(0, 0))
                nc.tensor.matmul(ps_pair[1][:, j, :], kt[64:128, ms],
                                 qt[64:128, lts],
                                 start=True, stop=True, tile_position=(64, 0))
            for i in range(2):
                nc.scalar.activation(
                    eh_lt[i][mg // 4][:, (mg % 4) * 2:(mg % 4) * 2 + 2, :],
                    ps_pair[i], AF.Exp)

        # prologue: only q + k token-tiles 0/1 (gated by the OWN-half LN)
        # run before the first scores; k2/k3 (other half) and all of V ride
        # the first head-chunk's stretches as fillers.
        w0 = load_wqk(0)
        qt, kt, units0 = qkproj_units(0, w0)
        for u in units0[:4]:
            u()
        prologue_rest = units0[4:]  # k2, k3
        cur = (qt, kt)
        pend_av = []  # AV units of (hc-1, lt1), scheduled into lt0 stretches

        for hc in range(NHC):
            if hc + 1 < NHC:
                w_n = load_wqk(hc + 1)
                qt_n, kt_n, units_n = qkproj_units(hc + 1, w_n)
            else:
                units_n = []
            qt, kt = cur
            eh = [[[None, None] for _ in range(2)] for _ in range(2)]
            for lt in range(2):
                if lt == 0:
                    fill = deque(pend_av)
                    pend_av = []
                    if hc == 0:
                        fill.extend(prologue_rest)  # k2, k3
                        fill.extend(v_units())      # all 16 V token-chunks
                        fill.extend(units_n[:2])
                    else:
                        fill.extend(units_n[:3])
                else:
                    fill = deque(av_units(hc, 0, eh[0]))
                    fill.extend(units_n[2:] if hc == 0 else units_n[3:])
                for mg in range(8):
                    scores_stretch(qt, kt, eh[lt], lt, mg)
                    if fill:
                        fill.popleft()()
                    if mg == 7:
                        while fill:
                            fill.popleft()()
            pend_av = av_units(hc, 1, eh[1])
            if hc + 1 < NHC:
                cur = (qt_n, kt_n)
        for u in pend_av:
            u()

    # ================= phase 4: proj + residual ================
    with ExitStack() as ph:
        sbp = ph.enter_context(tc.tile_pool(name="p4_sb", bufs=2))
        psp = ph.enter_context(tc.tile_pool(name="p4_ps", bufs=1, space="PSUM"))
        wpbig = sbp.tile([128, DC, D], bf16, tag="wp", bufs=1, name="wpbig")
        for dc in range(DC):
            nc.sync.dma_start(out=wpbig[:, dc, :],
                              in_=wproj[dc * 128:(dc + 1) * 128, :])
        for lt in range(NLT_OWN):
            for ft in range(DC):
                ps = psp.tile([128, LT], f32, tag="pj", bufs=4, name="ps_pj")
                for dc in range(DC):
                    nc.tensor.matmul(
                        ps, wpbig[:, dc, ft * 128:(ft + 1) * 128],
                        attn_sb[:, dc, lt * LT:(lt + 1) * LT],
                        start=(dc == 0), stop=(dc == DC - 1))
                gh = sbp.tile([128, LT], f32, tag="gh", bufs=3, name="gh")
                nc.scalar.activation(gh, ps, AF.Identity,
                                     bias=gmbp[:, ft:ft + 1],
                                     scale=gate_msa[:, ft:ft + 1])
                xo = x_own[:, ft, lt * LT:(lt + 1) * LT]
                nc.vector.tensor_tensor(xo, xo, gh, ALU.add)
    fr_attn()
    fr_wv()
    fr_bv()
    fr_xmod()
    fr_v()

    # ================= phase 5/6: LN2 + MLP ================
    outr = out.rearrange("(c p) t -> p c t", p=128)
    gelu_sb, fr_gelu = tc.tile([128, MC, LOWN], bf16, name="gelu_sb")
    with ExitStack() as ph:
        sbp = ph.enter_context(tc.tile_pool(name="p5_sb", bufs=2))
        psp = ph.enter_context(tc.tile_pool(name="p5_ps", bufs=1, space="PSUM"))
        h2mod = sbp.tile([128, DC, LOWN], bf16, tag="h2", bufs=1, name="h2mod")
        w1big = sbp.tile([128, DC, MLPD], bf16, tag="w1", bufs=1, name="w1big")
        for dc in range(DC):
            nc.sync.dma_start(out=w1big[:, dc, :],
                              in_=w1[dc * 128:(dc + 1) * 128, :])
        for lt in range(NLT_OWN):
            ln_tile(sbp, psp, x_own[:, :, lt * LT:(lt + 1) * LT],
                    h2mod[:, :, lt * LT:(lt + 1) * LT], s_mlp, shift_mlp,
                    bc_bufs=1)
        for ft in range(MC):
            # one [128, 2, 512] psum tile per ft: both token tiles, and
            # a single batched gelu over 1024 columns
            ps = psp.tile([128, NLT_OWN, LT], f32, tag="f1", bufs=2,
                          name="ps_f1")
            for dc in range(DC):
                wti = w1big[:, dc, ft * 128:(ft + 1) * 128]
                for lt in range(NLT_OWN):
                    nc.tensor.matmul(
                        ps[:, lt, :], wti,
                        h2mod[:, dc, lt * LT:(lt + 1) * LT],
                        start=(dc == 0), stop=(dc == DC - 1))
            nc.scalar.activation(gelu_sb[:, ft, :], ps, AF.Gelu,
                                 bias=bias_sb["b1"][:, ft:ft + 1])

    # fc2: ft-outer with resident w2 so each stationary serves both token
    # tiles (LDWEIGHTS amortized); fc1's pool is closed to make room.
    with ExitStack() as ph:
        sbp = ph.enter_context(tc.tile_pool(name="p7_sb", bufs=2))
        psp2 = ph.enter_context(tc.tile_pool(name="p7_ps", bufs=1,
                                             space="PSUM"))
        w2big = sbp.tile([128, MC, D], bf16, tag="w2", bufs=1, name="w2big")
        for mc in range(MC):
            nc.sync.dma_start(out=w2big[:, mc, :],
                              in_=w2[mc * 128:(mc + 1) * 128, :])
        for ft in range(DC):
            ps = [psp2.tile([128, LT], f32, tag=f"f2{i}", bufs=2,
                            name=f"ps_f2{i}") for i in range(NLT_OWN)]
            for mc in range(MC):
                wti = w2big[:, mc, ft * 128:(ft + 1) * 128]
                for lt in range(NLT_OWN):
                    nc.tensor.matmul(
                        ps[lt], wti, gelu_sb[:, mc, lt * LT:(lt + 1) * LT],
                        start=(mc == 0), stop=(mc == MC - 1))
            for lt in range(NLT_OWN):
                gh = sbp.tile([128, LT], f32, tag="gh2", bufs=3, name="gh2")
                nc.scalar.activation(gh, ps[lt], AF.Identity,
                                     bias=gmb2[:, ft:ft + 1],
                                     scale=gate_mlp[:, ft:ft + 1])
                xo = x_own[:, ft, lt * LT:(lt + 1) * LT]
                nc.vector.tensor_tensor(xo, xo, gh, ALU.add)
            # stream this feature chunk of the output right away
            nc.sync.dma_start(out=outr[:, ft, :], in_=x_own[:, ft, :])
    fr_gelu()

    # (output halves are streamed out at the end of each fc2 token tile)

    # release persistents in reverse creation order
    fr_x_own()
    fr_g2(); fr_g1(); fr_s2(); fr_s1(); fr_tp()
    for fr in reversed(bias_frees):
        fr()
    fr_eps(); fr_ones_bf(); fr_ones_f32()


_PROGRAM_CACHE = {}


def _get_program():
    if "nc" not in _PROGRAM_CACHE:
        _PROGRAM_CACHE["nc"] = build_program()
    return _PROGRAM_CACHE["nc"]


def _fm(v):
    """[D] vector -> feature-major [128, D//128] (partition p, chunk c)."""
    return np.ascontiguousarray(np.asarray(v, np.float32).reshape(-1, 128).T)


def make_in_maps(x, time_emb, Wqkv, bqkv, Wproj, bproj, W1, b1, W2, b2, Wt, bt,
                 g1, be1, g2, be2):
    # g1/be1/g2/be2 are identity layernorm params in this module; verify and
    # fold them away.
    assert np.allclose(g1, 1.0) and np.allclose(g2, 1.0)
    assert np.allclose(be1, 0.0) and np.allclose(be2, 0.0)

    x = np.asarray(x, np.float32)
    wqkv_f = np.asarray(Wqkv, np.float32)
    # interleave q-chunk hc / k-chunk hc column blocks of 128
    wqk_packed = np.empty((D, 2 * D), np.float32)
    for hc in range(DC):
        wqk_packed[:, hc * 256:hc * 256 + 128] = \
            wqkv_f[:, hc * 128:(hc + 1) * 128]
        wqk_packed[:, hc * 256 + 128:(hc + 1) * 256] = \
            wqkv_f[:, D + hc * 128:D + (hc + 1) * 128]
    shared = {
        "wqk": wqk_packed.astype(BF),
        "wv": np.ascontiguousarray(wqkv_f[:, 2 * D:]).astype(BF),
        "bq": _fm(np.asarray(bqkv[:D]) * 0.125),
        "bk": _fm(bqkv[D:2 * D]),
        "bv": np.ascontiguousarray(
            np.asarray(bqkv[2 * D:], np.float32)[None, :]).astype(BF),
        "wproj": np.asarray(Wproj, np.float32).astype(BF),
        "bproj": _fm(bproj),
        "w1": np.asarray(W1, np.float32).astype(BF),
        "b1": _fm(b1),
        "w2": np.asarray(W2, np.float32).astype(BF),
        "b2": _fm(b2),
        "wt": np.asarray(Wt, np.float32).astype(BF),
        "bt": _fm(bt),
    }
    in_maps = []
    for c in range(NCORES):
        b, half = c // 2, c % 2
        xb = x[b].T  # [D, L] feature-major
        own = slice(half * LOWN, (half + 1) * LOWN)
        oth = slice((1 - half) * LOWN, (2 - half) * LOWN)
        m = dict(shared)
        m["xfm"] = np.ascontiguousarray(
            np.concatenate([xb[:, own], xb[:, oth]], axis=1))
        m["temb"] = _fm(time_emb[b])
        in_maps.append(m)
    return in_maps


def assemble_output(results):
    outp = np.empty((B, L, D), np.float32)
    for c in range(NCORES):
        b, half = c // 2, c % 2
        outp[b, half * LOWN:(half + 1) * LOWN, :] = results[c]["out_fm"].T
    return outp


def kernel(x, time_emb, Wqkv, bqkv, Wproj, bproj, W1, b1, W2, b2, Wt, bt,
           g1, be1, g2, be2, trace=False, trace_kwargs=None, trace_cores=None):
    in_maps = make_in_maps(x, time_emb, Wqkv, bqkv, Wproj, bproj, W1, b1,
                           W2, b2, Wt, bt, g1, be1, g2, be2)
    nc = _get_program()
    res = run_bass_kernel_spmd(nc, in_maps, core_ids=list(range(NCORES)),
                               trace=trace, trace_kwargs=trace_kwargs or {},
                               trace_cores=trace_cores)
    kernel.last_results = res
    return assemble_output(res.results)
